# revision 1
# baseline (speedup 1.0000x reference)
"""3-layer GAT (DGL-style GATConv) on one TRN2 chip (8 NeuronCores).

Sharding: nodes are range-partitioned across the 8 cores (graph parallel).
Edges are bucketed by destination shard and sorted by destination; each core
owns the edge softmax + message aggregation for its node range.  Per layer,
each core computes its slice of the packed projection [feat | el | er]
(el/er attention dot products folded into the weight matrix on the host),
the slices are AllGather'ed, and per-edge source records are fetched from
the gathered table with indirect DMA.  Scatter-add into destinations is a
matmul with a 0/1 selector matrix built from an iota/is_equal compare.
"""

import os
import sys

import numpy as np

if "/opt/trn_rl_repo" not in sys.path:
    sys.path.insert(0, "/opt/trn_rl_repo")

import ml_dtypes

P = 128            # partitions / block size
NSH = 8            # shards (NeuronCores)
REC = 264          # record width in bf16 units: 256 feat bf16 + 4 el f32
RECF = REC // 2    # record width in f32 units
ELF = 128          # f32-unit offset of el inside a record
NB_MAX = 16        # edge blocks gathered per indirect DMA

# problem constants
N, E = 50000, 800000
IN_DIM, HID, HEADS, OUT_DIM = 512, 256, 4, 256
NEG_SLOPE = 0.2
NS = N // NSH                      # 6250 real nodes per shard
TILES = (NS + P - 1) // P          # 49
NSP = TILES * P                    # 6272 padded nodes per shard


def preprocess_edges(src, dst, ns, nsp, nsh):
    """Bucket edges by dst shard, sort by dst, tile into 128-node dst tiles,
    block into 128-edge blocks.  Block counts per tile are maxed across
    shards so all cores share one instruction stream.

    Returns (nb[t] per tile, B total blocks, and per-core [P, B] arrays:
    src row ids into the gathered table, local dst ids for the er gather,
    and dst-local-in-tile values (-1 for padding) for the selector compare).
    """
    tiles = nsp // P
    src = np.asarray(src).astype(np.int64)
    dst = np.asarray(dst).astype(np.int64)
    shard = dst // ns
    local = dst - shard * ns
    s_sh = src // ns
    prow = (s_sh * nsp + (src - s_sh * ns)).astype(np.int64)  # row in gathered table

    cnt = np.zeros((nsh, tiles), np.int64)
    np.add.at(cnt, (shard, local // P), 1)
    nb = np.maximum(1, -(-cnt // P)).max(axis=0).astype(np.int64)  # ceil, >=1
    B = int(nb.sum())
    base = np.zeros(tiles, np.int64)
    base[1:] = np.cumsum(nb)[:-1]

    src_idx = np.zeros((nsh, P, B), np.int32)
    dstloc = np.full((nsh, P, B), -1.0, np.float32)
    for c in range(nsh):
        m = shard == c
        loc_c = local[m]
        order = np.argsort(loc_c, kind="stable")
        loc_c = loc_c[order]
        prow_c = prow[m][order]
        tile_c = loc_c // P
        for t in range(tiles):
            sel = tile_c == t
            k = int(sel.sum())
            if k == 0:
                continue
            j = np.arange(k)
            pp = j % P
            bb = base[t] + j // P
            src_idx[c, pp, bb] = prow_c[sel]
            dstloc[c, pp, bb] = (loc_c[sel] - t * P).astype(np.float32)
    # dstloc_row: block-major edge-order [B*P]: entry b*P+p = dstloc[p, b]
    dstloc_row = np.ascontiguousarray(dstloc.transpose(0, 2, 1)).reshape(
        nsh, B * P)
    return nb, base, B, src_idx, dstloc, dstloc_row


def pack_weights(W, al, ar):
    """[W | W@blockdiag(al) | W@blockdiag(ar) | zero-pad] -> [k, REC] f32."""
    W = np.asarray(W, np.float32)
    al = np.asarray(al, np.float32)
    ar = np.asarray(ar, np.float32)
    H, D = al.shape
    k = W.shape[0]
    W3 = W.reshape(k, H, D)
    Wel = np.einsum("khd,hd->kh", W3, al)
    Wer = np.einsum("khd,hd->kh", W3, ar)
    pad = np.zeros((k, REC - 256 - 2 * H), np.float32)
    return np.concatenate([W, Wel, Wer, pad], axis=1)


def build_bass(nsp, in_dim, nb, base, B, heads):
    """Build the 3-layer SPMD Bass graph (one graph, 8 cores)."""
    from contextlib import ExitStack

    import concourse.bacc as bacc
    import concourse.bass as bass
    import concourse.mybir as mybir
    import concourse.tile as tile
    from concourse.bass import AP, IndirectOffsetOnAxis
    from concourse.masks import make_identity

    dt = mybir.dt
    f32, bf16, i32 = dt.float32, dt.bfloat16, dt.int32
    AF = mybir.ActivationFunctionType
    tiles = nsp // P
    kdims = [in_dim, 256, 256]

    nc = bacc.Bacc("TRN2", target_bir_lowering=False, debug=False,
                   num_devices=NSH)

    h0T = nc.dram_tensor("h0T", [in_dim, nsp], bf16, kind="ExternalInput")
    wps = [nc.dram_tensor(f"wpack{l}", [kdims[l], REC], bf16,
                          kind="ExternalInput") for l in range(3)]
    bias_d = nc.dram_tensor("biases", [3, 256], f32, kind="ExternalInput")
    src_idx_d = nc.dram_tensor("src_idx", [P, B], i32, kind="ExternalInput")
    dstloc_d = nc.dram_tensor("dstloc", [P, B], bf16, kind="ExternalInput")
    dstrow_d = nc.dram_tensor("dstrow", [1, B * P], bf16,
                              kind="ExternalInput")
    out_d = nc.dram_tensor("out", [nsp, 256], f32, kind="ExternalOutput")

    p_slice = [nc.dram_tensor(f"pslice{l}", [nsp, REC], bf16)
               for l in range(3)]
    p_full = [nc.dram_tensor(f"pfull{l}", [NSH * nsp, REC], bf16,
                             addr_space="Shared") for l in range(3)]

    NBH = int(max(nb))
    with tile.TileContext(nc) as tc, ExitStack() as ctx:
        const = ctx.enter_context(tc.tile_pool(name="const", bufs=1))
        psum_pk = ctx.enter_context(
            tc.tile_pool(name="psum_pk", bufs=2, space="PSUM"))
        psum_ms = ctx.enter_context(
            tc.tile_pool(name="psum_ms", bufs=2, space="PSUM"))
        psum_er = ctx.enter_context(
            tc.tile_pool(name="psum_er", bufs=2, space="PSUM"))
        psum_tr = ctx.enter_context(
            tc.tile_pool(name="psum_tr", bufs=2, space="PSUM"))
        gpool = ctx.enter_context(tc.tile_pool(name="gpool", bufs=2))
        selp = ctx.enter_context(tc.tile_pool(name="selp", bufs=2))
        rpool = ctx.enter_context(tc.tile_pool(name="rpool", bufs=2))
        spool = ctx.enter_context(tc.tile_pool(name="spool", bufs=4))
        mpool = ctx.enter_context(tc.tile_pool(name="mpool", bufs=2))

        # constants / persistent state
        iota_i = const.tile([P, P], i32, name="iota_i", tag="iota_i")
        nc.gpsimd.iota(iota_i[:], pattern=[[1, P]], base=0,
                       channel_multiplier=0)
        iota_bf = const.tile([P, P], bf16, name="iota_bf", tag="iota_bf")
        nc.vector.tensor_copy(iota_bf[:], iota_i[:])
        iotac_i = const.tile([P, 1], i32, name="iotac_i", tag="iotac_i")
        nc.gpsimd.iota(iotac_i[:], pattern=[[1, 1]], base=0,
                       channel_multiplier=1)
        iotac_bf = const.tile([P, 1], bf16, name="iotac_bf", tag="iotac_bf")
        nc.vector.tensor_copy(iotac_bf[:], iotac_i[:])
        ident = const.tile([P, P], bf16, name="ident", tag="ident")
        make_identity(nc, ident[:])

        src_idx_sb = const.tile([P, B], i32, name="srcidx", tag="srcidx")
        nc.sync.dma_start(src_idx_sb[:], src_idx_d[:, :])
        dstloc_sb = const.tile([P, B], bf16, name="dstloc", tag="dstloc")
        nc.sync.dma_start(dstloc_sb[:], dstloc_d[:, :])

        h_in0 = [const.tile([P, nsp], bf16, name=f"h0_{k}", tag=f"h0_{k}")
                 for k in range(in_dim // P)]
        for k in range(in_dim // P):
            nc.sync.dma_start(h_in0[k][:], h0T[k * P:(k + 1) * P, :])
        hT1 = [const.tile([P, nsp], bf16, name=f"h1_{k}", tag=f"h1_{k}")
               for k in range(2)]
        hT2 = [const.tile([P, nsp], bf16, name=f"h2_{k}", tag=f"h2_{k}")
               for k in range(2)]
        h_ins = [h_in0, hT1, hT2]
        h_outs = [hT1, hT2, None]

        for l in range(3):
            H = heads[l]
            DH = 256 // H
            CH = 256 + H           # scatter-matmul rhs cols: [sum | msg]
            kch = kdims[l] // P
            h_in = h_ins[l]
            h_out = h_outs[l]

            w_sb = [const.tile([P, REC], bf16, name=f"w{l}_{k}",
                               tag=f"w{l}_{k}") for k in range(kch)]
            for k in range(kch):
                nc.sync.dma_start(w_sb[k][:], wps[l][k * P:(k + 1) * P, :])
            b_tile = const.tile([P, 256], f32, name=f"btile{l}",
                                tag=f"btile{l}")
            nc.sync.dma_start(b_tile[:],
                              bias_d[l:l + 1, :].to_broadcast((P, 256)))
            er_all = const.tile([P, tiles * H], bf16, name=f"erall{l}",
                                tag=f"erall{l}")

            # ---- pack phase: [feat | el | er] = h @ wpack ----
            for t in range(tiles):
                ps = psum_pk.tile([P, REC], f32, name="pspk", tag="pspk")
                for k in range(kch):
                    nc.tensor.matmul(
                        ps[:], lhsT=h_in[k][:, t * P:(t + 1) * P],
                        rhs=w_sb[k][:], start=(k == 0), stop=(k == kch - 1))
                pack = gpool.tile([P, REC], bf16, name="pack", tag="pack")
                nc.vector.tensor_copy(pack[:, 0:256], ps[:, 0:256])
                pf = pack[:].bitcast(f32)
                el_dst = AP(pf.tensor, pf.offset + ELF, [pf.ap[0], [1, 4]])
                nc.vector.tensor_copy(el_dst, ps[:, 256:260])
                nc.vector.tensor_copy(er_all[:, t * H:(t + 1) * H],
                                      ps[:, 256 + H:256 + 2 * H])
                nc.sync.dma_start(p_slice[l][t * P:(t + 1) * P, :], pack[:])

            # ---- allgather the packed table ----
            nc.gpsimd.collective_compute(
                "AllGather", mybir.AluOpType.bypass,
                replica_groups=[list(range(NSH))],
                ins=[p_slice[l].ap().opt()], outs=[p_full[l].ap().opt()])

            # ---- edge phase ----
            for t in range(tiles):
                nblk = int(nb[t])
                g0 = int(base[t])
                ps_m = psum_ms.tile([P, CH], f32, name="psms", tag="psms")

                # gather source records, one 128-edge block per indirect DMA
                G = gpool.tile([P, NBH * REC], bf16, name="G", tag="G")
                for j in range(nblk):
                    nc.gpsimd.indirect_dma_start(
                        out=G[:, j * REC:(j + 1) * REC], out_offset=None,
                        in_=p_full[l][:, :],
                        in_offset=IndirectOffsetOnAxis(
                            ap=src_idx_sb[:, g0 + j:g0 + j + 1], axis=0))

                # er broadcast: replicate dstloc row, compare vs column iota,
                # then one Nf=H matmul per block against this tile's er rows
                rep = rpool.tile([P, NBH * P], bf16, name="rep", tag="rep")
                nc.sync.dma_start(
                    rep[:, 0:nblk * P],
                    dstrow_d[0:1, g0 * P:(g0 + nblk) * P].to_broadcast(
                        (P, nblk * P)))
                msel = selp.tile([P, NBH * P], bf16, name="msel", tag="msel")
                ioc = iotac_bf[:]
                in1c = AP(ioc.tensor, ioc.offset, [ioc.ap[0], [0, nblk * P]])
                nc.vector.tensor_tensor(out=msel[:, 0:nblk * P],
                                        in0=rep[:, 0:nblk * P], in1=in1c,
                                        op=mybir.AluOpType.is_equal)
                ps_er = psum_er.tile([P, NBH * H], f32, name="pser",
                                     tag="pser")
                for j in range(nblk):
                    nc.tensor.matmul(
                        ps_er[:, j * H:(j + 1) * H],
                        lhsT=msel[:, j * P:(j + 1) * P],
                        rhs=er_all[:, t * H:(t + 1) * H],
                        start=True, stop=True)

                # e = lrelu(el + er); t = exp(e)
                gap = G[:]
                gf = gap.bitcast(f32)
                el_ap = AP(gf.tensor, gf.offset + ELF,
                           [gf.ap[0], [RECF, nblk], [1, H]])
                er3 = ps_er[:, 0:nblk * H].rearrange("p (b h) -> p b h", h=H)
                e1 = spool.tile([P, NBH * H], f32, name="e1", tag="e1")
                e13 = e1[:, 0:nblk * H].rearrange("p (b h) -> p b h", h=H)
                nc.vector.tensor_tensor(out=e13, in0=el_ap, in1=er3,
                                        op=mybir.AluOpType.add)
                e2 = spool.tile([P, NBH * H], f32, name="e2", tag="e2")
                nc.vector.tensor_scalar_mul(
                    e2[:, 0:nblk * H], e1[:, 0:nblk * H], NEG_SLOPE)
                e3 = spool.tile([P, NBH * H], f32, name="e3", tag="e3")
                nc.vector.tensor_tensor(out=e3[:, 0:nblk * H],
                                        in0=e1[:, 0:nblk * H],
                                        in1=e2[:, 0:nblk * H],
                                        op=mybir.AluOpType.max)
                t_bf = spool.tile([P, NBH * H], bf16, name="tbf", tag="tbf")
                nc.scalar.activation(t_bf[:, 0:nblk * H], e3[:, 0:nblk * H],
                                     AF.Exp)

                # scatter selector: dstloc column vs row iota
                sel = selp.tile([P, NBH * P], bf16, name="sel", tag="sel")
                dl = dstloc_sb[:]
                in0 = AP(dl.tensor, dl.offset + g0,
                         [dl.ap[0], [1, nblk], [0, P]])
                io = iota_bf[:]
                in1 = AP(io.tensor, io.offset, [io.ap[0], [0, nblk], [1, P]])
                sel3 = sel[:, 0:nblk * P].rearrange("p (b q) -> p b q", q=P)
                nc.vector.tensor_tensor(out=sel3, in0=in0, in1=in1,
                                        op=mybir.AluOpType.is_equal)

                # rhs = [t | t * feat] per block
                rhs = rpool.tile([P, NBH * CH], bf16, name="rhs", tag="rhs")
                rap = rhs[:]
                t3 = t_bf[:, 0:nblk * H].rearrange("p (b h) -> p b h", h=H)
                s_dst = AP(rap.tensor, rap.offset,
                           [rap.ap[0], [CH, nblk], [1, H]])
                nc.vector.tensor_copy(s_dst, t3)
                gfeat = AP(gap.tensor, gap.offset,
                           [gap.ap[0], [REC, nblk], [DH, H], [1, DH]])
                tb = t_bf[:]
                tmul = AP(tb.tensor, tb.offset,
                          [tb.ap[0], [H, nblk], [1, H], [0, DH]])
                r_dst = AP(rap.tensor, rap.offset + H,
                           [rap.ap[0], [CH, nblk], [DH, H], [1, DH]])
                nc.vector.tensor_tensor(out=r_dst, in0=gfeat, in1=tmul,
                                        op=mybir.AluOpType.mult)

                for j in range(nblk):
                    nc.tensor.matmul(
                        ps_m[:], lhsT=sel[:, j * P:(j + 1) * P],
                        rhs=rhs[:, j * CH:(j + 1) * CH],
                        start=(j == 0), stop=(j == nblk - 1))

                # ---- tile epilogue: msg / sum + bias (+relu, transpose) ----
                s_sb = spool.tile([P, H], f32, name="ssb", tag="ssb")
                nc.vector.tensor_scalar_max(s_sb[:], ps_m[:, 0:H], 1e-30)
                r_sb = spool.tile([P, H], f32, name="rsb", tag="rsb")
                nc.vector.reciprocal(r_sb[:], s_sb[:])
                mn = mpool.tile([P, 256], f32, name="mn", tag="mn")
                mn3 = mn[:].rearrange("p (h d) -> p h d", h=H)
                ms3 = ps_m[:, H:H + 256].rearrange("p (h d) -> p h d", h=H)
                rb = r_sb[:]
                r_bc = AP(rb.tensor, rb.offset, [rb.ap[0], [1, H], [0, DH]])
                nc.vector.tensor_tensor(out=mn3, in0=ms3, in1=r_bc,
                                        op=mybir.AluOpType.mult)
                mb = mpool.tile([P, 256], f32, name="mb", tag="mb")
                nc.vector.tensor_tensor(out=mb[:], in0=mn[:], in1=b_tile[:],
                                        op=mybir.AluOpType.add)
                if l < 2:
                    hb = mpool.tile([P, 256], bf16, name="hb", tag="hb")
                    nc.scalar.activation(hb[:], mb[:], AF.Relu)
                    for k in range(2):
                        pt = psum_tr.tile([P, P], bf16, name="pstr",
                                          tag="pstr")
                        nc.tensor.transpose(pt[:], hb[:, k * P:(k + 1) * P],
                                            ident[:])
                        nc.vector.tensor_copy(
                            h_out[k][:, t * P:(t + 1) * P], pt[:])
                else:
                    nc.sync.dma_start(out_d[t * P:(t + 1) * P, :], mb[:])

    nc.compile()
    return nc


def _make_in_maps(feats, wpacks, biases, nb, base, B,
                  src_idx, dstloc, dstloc_row, ns, nsp, in_dim):
    bf = ml_dtypes.bfloat16
    in_maps = []
    for c in range(NSH):
        sl = np.zeros((nsp, in_dim), np.float32)
        sl[:ns] = feats[c * ns:(c + 1) * ns]
        in_maps.append({
            "h0T": np.ascontiguousarray(sl.T).astype(bf),
            "wpack0": wpacks[0].astype(bf),
            "wpack1": wpacks[1].astype(bf),
            "wpack2": wpacks[2].astype(bf),
            "biases": biases.astype(np.float32),
            "src_idx": np.ascontiguousarray(src_idx[c]),
            "dstloc": dstloc[c].astype(bf),
            "dstrow": dstloc_row[c].reshape(1, -1).astype(bf),
        })
    return in_maps


def gat_host(feats, src, dst, W0, al0, ar0, b0, W1, al1, ar1, b1,
             W2, al2, ar2, b2, ns=NS, nsp=NSP, in_dim=IN_DIM, run=None):
    """Full host flow: preprocess, build, run (via `run` callback), unshard."""
    feats = np.asarray(feats, np.float32)
    heads = [al0.shape[0], al1.shape[0], al2.shape[0]]
    wpacks = [pack_weights(W0, al0, ar0), pack_weights(W1, al1, ar1),
              pack_weights(W2, al2, ar2)]
    biases = np.stack([np.asarray(b0, np.float32),
                       np.asarray(b1, np.float32),
                       np.asarray(b2, np.float32)])
    nb, base, B, src_idx, dstloc, dstloc_row = preprocess_edges(
        src, dst, ns, nsp, NSH)
    nc = build_bass(nsp, in_dim, nb, base, B, heads)
    in_maps = _make_in_maps(feats, wpacks, biases, nb, base, B,
                            src_idx, dstloc, dstloc_row, ns, nsp, in_dim)
    results = run(nc, in_maps)
    out = np.concatenate([results[c]["out"][:ns] for c in range(NSH)], axis=0)
    return np.ascontiguousarray(out.astype(np.float32))


def kernel(**inputs):
    from concourse.bass_utils import run_bass_kernel_spmd

    trace = os.environ.get("GAT_TRACE", "0") == "1"
    tmpdir = os.environ.get("GAT_TRACE_DIR") or None

    def run(nc, in_maps):
        res = run_bass_kernel_spmd(nc, in_maps, core_ids=list(range(NSH)),
                                   trace=trace, tmpdir=tmpdir)
        if trace:
            print(f"HW exec time: {res.exec_time_ns} ns")
        return res.results

    return gat_host(
        inputs["feats"], inputs["src"], inputs["dst"],
        inputs["W0"], inputs["al0"], inputs["ar0"], inputs["b0"],
        inputs["W1"], inputs["al1"], inputs["ar1"], inputs["b1"],
        inputs["W2"], inputs["al2"], inputs["ar2"], inputs["b2"],
        run=run)



# revision 17
# speedup vs baseline: 1.6333x; 1.6333x over previous
"""3-layer GAT (DGL-style GATConv) on one TRN2 chip (8 NeuronCores).

Sharding: nodes are range-partitioned across the 8 cores (graph parallel).
Within a core, nodes are permuted by descending in-degree and assigned to
(tile, partition) slots so that every destination node owns one partition
row: edge slot (p, b) of tile t holds the b-th incoming edge of the node at
rank t*128+p.  This makes the edge softmax + aggregation selector-free:

  - er broadcast is a stride-0 AP read of the tile's own er column,
  - the weighted-message reduction over b is a chain of identity-stationary
    matmuls accumulating in PSUM (the exp(t) values ride along as H extra
    columns and produce the softmax denominators in the same pass),
  - padded slots gather a sentinel record whose el is -1e30 so exp()==0.

Per layer, each core packs [feat | el | er] = h @ wpack for its own nodes
(attention dots folded into the weight matrix host-side, feat columns stored
d-major so the t*feat multiply runs in the DVE 2x perf mode), the slices are
AllGather'ed, and per-edge source records are fetched from the gathered
table with one batched indirect DMA per destination tile.
"""

import os
import sys

import numpy as np

if "/opt/trn_rl_repo" not in sys.path:
    sys.path.insert(0, "/opt/trn_rl_repo")

import ml_dtypes

P = 128            # partitions / block size
NSH = 8            # shards (NeuronCores)
REC = 384          # record stride in bf16 units (768B, dma_gather needs
                   # a 256B multiple): 256 feat bf16 + 4 el f32 + pad
RECF = REC // 2    # record width in f32 units
ELF = 128          # f32-unit offset of el inside a record
PKW = 264          # packed-projection width: [feat 256 | el H | er H]

# problem constants
N, E = 50000, 800000
IN_DIM, HID, HEADS, OUT_DIM = 512, 256, 4, 256
NEG_SLOPE = 0.2
NS = N // NSH                      # 6250 real nodes per shard
TILES = (NS + P - 1) // P          # 49
NSP = TILES * P                    # 6272 padded nodes per shard
MID = NSH * NSP // 2               # gather table base row: int16 idx covers
                                   # rows MID-32768 .. MID+32767 (whole table)
SENT_IDX = NSP - 1                 # = row 4*NSP+(NSP-1) - MID: core 4's
                                   # lowest-degree rank, a padding node whose
                                   # el is forced to -1e30 (positive idx, so
                                   # sentinel-padded chunk tails never trip
                                   # the ucode's trailing-negative truncation)
CHUNK = 8                          # blocks per dma_gather: 1024 descriptors
                                   # fills the SWDGE ring exactly


def _chunks(nblk):
    return [(jb, min(jb + CHUNK, nblk)) for jb in range(0, nblk, CHUNK)]


def _reserved(nblk):
    """Slot j values on partition 127 kept as sentinel (each chunk's final
    stream slot must hold a non-negative index)."""
    return {je - 1 for _, je in _chunks(nblk)}


def preprocess_edges(src, dst, ns=NS, nsp=NSP, nsh=NSH):
    """Bucket edges by dst shard; per shard, rank nodes by descending
    in-degree (rank = tile*128 + partition).  Edge slot (p, base[t]+b) holds
    the b-th incoming edge of rank t*128+p; unused slots point at SENT_IDX.

    Returns (nb[t] blocks per tile, base, B total blocks, pos[nsh, nsp]
    node->rank maps, idx16[nsh, P, B*8] per-chunk 16-partition-wrapped int16
    indices (row - MID) into the gathered table)."""
    tiles = nsp // P
    src = np.asarray(src).astype(np.int64)
    dst = np.asarray(dst).astype(np.int64)
    s_sh = src // ns
    s_loc = src - s_sh * ns
    d_sh = dst // ns
    d_loc = dst - d_sh * ns

    pos = np.zeros((nsh, nsp), np.int64)
    deg_rank = np.zeros((nsh, nsp), np.int64)
    for c in range(nsh):
        deg = np.bincount(d_loc[d_sh == c], minlength=ns)
        degf = np.concatenate([deg, np.zeros(nsp - ns, np.int64)])
        order = np.argsort(-degf, kind="stable")
        p = np.empty(nsp, np.int64)
        p[order] = np.arange(nsp)
        pos[c] = p
        deg_rank[c] = degf[order]

    maxdeg = np.maximum(1, deg_rank[:, ::P].max(axis=0))
    deg127 = deg_rank[:, P - 1::P].max(axis=0)
    nb = np.zeros(tiles, np.int64)
    for t in range(tiles):
        m = int(maxdeg[t])
        while m - len(_reserved(m)) < int(deg127[t]):
            m += 1
        nb[t] = m
    base = np.zeros(tiles, np.int64)
    base[1:] = np.cumsum(nb)[:-1]
    B = int(nb.sum())

    src_row = (s_sh * nsp + pos[s_sh, s_loc]).astype(np.int64)
    vals = np.full((nsh, P, B), SENT_IDX, np.int16)
    allowed127 = [np.array(sorted(set(range(int(nb[t])))
                                  - _reserved(int(nb[t]))), np.int64)
                  for t in range(tiles)]
    for c in range(nsh):
        m = d_sh == c
        r = pos[c, d_loc[m]]
        sr = (src_row[m] - MID).astype(np.int16)
        o = np.argsort(r, kind="stable")
        rs, srs = r[o], sr[o]
        b = np.arange(len(rs)) - np.searchsorted(rs, rs)
        t_arr, p_arr = rs // P, rs % P
        j_arr = b.copy()
        is127 = p_arr == P - 1
        for t in range(tiles):
            sel = is127 & (t_arr == t)
            if sel.any():
                j_arr[sel] = allowed127[t][b[sel]]
        vals[c, p_arr, base[t_arr] + j_arr] = srs

    # wrap each gather chunk's slot stream (i = j*128 + p) into the 16-row
    # replicated int16 layout: value for stream slot i goes to
    # [16*g + i % 16, colbase + i // 16] for every gpsimd core g
    idx16 = np.zeros((nsh, P, B * 8), np.int16)
    for t in range(tiles):
        for jb, je in _chunks(int(nb[t])):
            blk = vals[:, :, base[t] + jb:base[t] + je]      # [nsh, P, nbk]
            stream = blk.transpose(0, 2, 1).reshape(nsh, -1)  # [nsh, i]
            w = stream.reshape(nsh, -1, 16).transpose(0, 2, 1)
            colb = int(base[t] + jb) * 8
            for g in range(8):
                idx16[:, 16 * g:16 * (g + 1), colb:colb + w.shape[2]] = w
            assert (stream[:, -1] >= 0).all(), "chunk tail must be >= 0"
    return nb, base, B, pos, idx16


def _dmaj(H, width=256):
    """Column permutation old(h-major) -> new(d-major): new j holds head
    j%H, dim j//H."""
    j = np.arange(width)
    return (j % H) * (width // H) + j // H


def pack_weights(W, al, ar, in_perm=None):
    """[W(d-major cols) | W@blockdiag(al) | W@blockdiag(ar)] -> [k, REC] f32.
    in_perm permutes W's rows to match the previous layer's d-major output."""
    W = np.asarray(W, np.float32)
    al = np.asarray(al, np.float32)
    ar = np.asarray(ar, np.float32)
    H, D = al.shape
    if in_perm is not None:
        W = W[in_perm]
    k = W.shape[0]
    W3 = W.reshape(k, H, D)
    Wel = np.einsum("khd,hd->kh", W3, al)
    Wer = np.einsum("khd,hd->kh", W3, ar)
    pad = np.zeros((k, PKW - 256 - 2 * H), np.float32)
    return np.concatenate([W[:, _dmaj(H)], Wel, Wer, pad], axis=1)


def build_bass(nsp, in_dim, nb, base, B, heads):
    """Build the 3-layer SPMD Bass graph (one graph, 8 cores)."""
    from contextlib import ExitStack

    import concourse.bacc as bacc
    import concourse.bass as bass
    import concourse.mybir as mybir
    import concourse.tile as tile
    from concourse.bass import AP, IndirectOffsetOnAxis
    from concourse.masks import make_identity

    dt = mybir.dt
    f32, bf16, i32 = dt.float32, dt.bfloat16, dt.int32
    AF = mybir.ActivationFunctionType
    tiles = nsp // P
    kdims = [in_dim, 256, 256]
    NBH = int(max(nb))

    i16 = dt.int16
    nc = bacc.Bacc("TRN2", target_bir_lowering=False, debug=False,
                   num_devices=NSH, num_swdge_queues=2)

    h0T = nc.dram_tensor("h0T", [in_dim, nsp], bf16, kind="ExternalInput")
    wps = [nc.dram_tensor(f"wpack{l}", [kdims[l], PKW], bf16,
                          kind="ExternalInput") for l in range(3)]
    bias_d = nc.dram_tensor("biases", [3, 256], f32, kind="ExternalInput")
    sent_d = nc.dram_tensor("sentinel", [1, 8], bf16, kind="ExternalInput")
    src_idx_d = nc.dram_tensor("src_idx", [P, B * 8], i16,
                               kind="ExternalInput")
    out_d = nc.dram_tensor("out", [nsp, 256], f32, kind="ExternalOutput")

    p_slice = [nc.dram_tensor(f"pslice{l}", [nsp, REC], bf16)
               for l in range(3)]
    p_full = [nc.dram_tensor(f"pfull{l}", [NSH * nsp, REC], bf16,
                             addr_space="Shared") for l in range(3)]
    hT_dram = [None,
               nc.dram_tensor("hT1", [256, nsp], bf16),
               nc.dram_tensor("hT2", [256, nsp], bf16)]

    with tile.TileContext(nc) as tc, ExitStack() as ctx:
        const = ctx.enter_context(tc.tile_pool(name="const", bufs=1))
        psum_pk = ctx.enter_context(
            tc.tile_pool(name="psum_pk", bufs=2, space="PSUM"))
        psum_ms = ctx.enter_context(
            tc.tile_pool(name="psum_ms", bufs=2, space="PSUM"))
        psum_tr = ctx.enter_context(
            tc.tile_pool(name="psum_tr", bufs=2, space="PSUM"))
        gpool = ctx.enter_context(tc.tile_pool(name="gpool", bufs=2))
        rpool = ctx.enter_context(tc.tile_pool(name="rpool", bufs=2))
        hpool = ctx.enter_context(tc.tile_pool(name="hpool", bufs=2))
        ppool = ctx.enter_context(tc.tile_pool(name="ppool", bufs=2))
        spool = ctx.enter_context(tc.tile_pool(name="spool", bufs=4))
        mpool = ctx.enter_context(tc.tile_pool(name="mpool", bufs=2))
        tpool = ctx.enter_context(tc.tile_pool(name="tpool", bufs=2))

        ident = const.tile([P, P], bf16, name="ident", tag="ident")
        make_identity(nc, ident[:])
        src_idx_sb = const.tile([P, B * 8], i16, name="srcidx", tag="srcidx")
        nc.sync.dma_start(src_idx_sb[:], src_idx_d[:, :])
        qn = [0]

        for l in range(3):
            H = heads[l]
            DH = 256 // H
            CH = 256 + H           # fold columns: [sum | msg]
            kch = kdims[l] // P

            w_sb = [const.tile([P, PKW], bf16, name=f"w{l}_{k}",
                               tag=f"w{l}_{k}") for k in range(kch)]
            for k in range(kch):
                nc.sync.dma_start(w_sb[k][:], wps[l][k * P:(k + 1) * P, :])
            b_tile = const.tile([P, 256], f32, name=f"btile{l}",
                                tag=f"btile{l}")
            nc.sync.dma_start(b_tile[:],
                              bias_d[l:l + 1, :].to_broadcast((P, 256)))
            er_all = const.tile([P, tiles * H], f32, name=f"erall{l}",
                                tag=f"erall{l}")
            hsrc = h0T if l == 0 else hT_dram[l]

            # ---- pack phase: [feat | el | er] = h @ wpack ----
            for t in range(tiles):
                hch = hpool.tile([P, kch * P], bf16, name="hch", tag="hch")
                hap = hsrc[:, t * P:(t + 1) * P]
                nc.sync.dma_start(
                    hch[:].rearrange("p (k q) -> p k q", k=kch),
                    AP(hap.tensor, hap.offset,
                       [[nsp, P], [P * nsp, kch], [1, P]]))
                ps = psum_pk.tile([P, PKW], f32, name="pspk", tag="pspk")
                for k in range(kch):
                    nc.tensor.matmul(
                        ps[:], lhsT=hch[:, k * P:(k + 1) * P],
                        rhs=w_sb[k][:], start=(k == 0), stop=(k == kch - 1))
                pack = ppool.tile([P, REC], bf16, name="pack", tag="pack")
                nc.scalar.activation(pack[:, 0:256], ps[:, 0:256], AF.Copy)
                pf = pack[:].bitcast(f32)
                el_dst = AP(pf.tensor, pf.offset + ELF, [pf.ap[0], [1, H]])
                nc.vector.tensor_copy(el_dst, ps[:, 256:256 + H])
                nc.vector.tensor_copy(er_all[:, t * H:(t + 1) * H],
                                      ps[:, 256 + H:256 + 2 * H])
                nc.sync.dma_start(p_slice[l][t * P:(t + 1) * P, :], pack[:])
            # sentinel record: el = -1e30 so exp(lrelu(el+er)) == 0
            nc.sync.dma_start(p_slice[l][nsp - 1:nsp, 256:256 + 2 * H],
                              sent_d[0:1, 0:2 * H])

            # ---- allgather the packed table ----
            nc.gpsimd.collective_compute(
                "AllGather", mybir.AluOpType.bypass,
                replica_groups=[list(range(NSH))],
                ins=[p_slice[l].ap().opt()], outs=[p_full[l].ap().opt()])

            # ---- edge phase ----
            for t in range(tiles):
                nblk = int(nb[t])
                g0 = int(base[t])

                G = gpool.tile([P, NBH * REC], bf16, name="G", tag="G")
                for jb, je in _chunks(nblk):
                    nidx = (je - jb) * P
                    colb = (g0 + jb) * 8
                    nc.gpsimd.dma_gather(
                        out_ap=G[:, jb * REC:je * REC].rearrange(
                            "p (b e) -> p b e", e=REC),
                        in_ap=p_full[l][MID:NSH * nsp, :],
                        idxs_ap=src_idx_sb[:, colb:colb + nidx // 16],
                        num_idxs=nidx,
                        num_idxs_reg=nidx,
                        elem_size=REC,
                        queue_num=qn[0])
                    qn[0] ^= 1

                # e = lrelu(el + er); t = exp(e)
                gf = G[:].bitcast(f32)
                el_ap = AP(gf.tensor, gf.offset + ELF,
                           [gf.ap[0], [RECF, nblk], [1, H]])
                ea = er_all[:]
                er_ap = AP(ea.tensor, ea.offset + t * H,
                           [ea.ap[0], [0, nblk], [1, H]])
                e1 = spool.tile([P, NBH * H], f32, name="e1", tag="e1")
                e13 = e1[:, 0:nblk * H].rearrange("p (b h) -> p b h", h=H)
                nc.vector.tensor_tensor(out=e13, in0=el_ap, in1=er_ap,
                                        op=mybir.AluOpType.add)
                e2 = spool.tile([P, NBH * H], f32, name="e2", tag="e2")
                nc.vector.tensor_scalar_mul(
                    e2[:, 0:nblk * H], e1[:, 0:nblk * H], NEG_SLOPE)
                e3 = spool.tile([P, NBH * H], f32, name="e3", tag="e3")
                nc.vector.tensor_tensor(out=e3[:, 0:nblk * H],
                                        in0=e1[:, 0:nblk * H],
                                        in1=e2[:, 0:nblk * H],
                                        op=mybir.AluOpType.max)
                t_bf = spool.tile([P, NBH * 4], bf16, name="tbf", tag="tbf")
                nc.scalar.activation(t_bf[:, 0:nblk * H], e3[:, 0:nblk * H],
                                     AF.Exp)
                tb = t_bf[:]
                if H == 1:
                    # replicate t x4 so the big multiply gets an innermost
                    # unit-stride run (DVE 2x perf mode)
                    t4 = spool.tile([P, NBH * 4], bf16, name="t4", tag="t4")
                    nc.vector.tensor_copy(
                        t4[:, 0:nblk * 4].rearrange("p (b r) -> p b r", r=4),
                        AP(tb.tensor, tb.offset, [tb.ap[0], [1, nblk],
                                                  [0, 4]]))
                    tin, tw = t4[:], 4
                else:
                    tin, tw = tb, H

                # rhs = [t | t * feat] per block (feat d-major)
                rhs = rpool.tile([P, NBH * CH], bf16, name="rhs", tag="rhs")
                rap = rhs[:]
                nc.vector.tensor_copy(
                    AP(rap.tensor, rap.offset, [rap.ap[0], [CH, nblk],
                                                [1, H]]),
                    AP(tb.tensor, tb.offset, [tb.ap[0], [H, nblk], [1, H]]))
                gap = G[:]
                nd = 256 // tw
                nc.vector.tensor_tensor(
                    out=AP(rap.tensor, rap.offset + H,
                           [rap.ap[0], [CH, nblk], [tw, nd], [1, tw]]),
                    in0=AP(gap.tensor, gap.offset,
                           [gap.ap[0], [REC, nblk], [tw, nd], [1, tw]]),
                    in1=AP(tin.tensor, tin.offset,
                           [tin.ap[0], [tw, nblk], [0, nd], [1, tw]]),
                    op=mybir.AluOpType.mult)

                # fold blocks: PSUM-accumulating identity matmuls
                ps_m = psum_ms.tile([P, CH], f32, name="psms", tag="psms")
                for j in range(nblk):
                    nc.tensor.matmul(
                        ps_m[:], lhsT=ident[:],
                        rhs=rhs[:, j * CH:(j + 1) * CH],
                        start=(j == 0), stop=(j == nblk - 1))

                # ---- tile epilogue: msg / sum + bias (+relu, transpose) ----
                s_sb = spool.tile([P, 4], f32, name="ssb", tag="ssb")
                nc.vector.tensor_scalar_max(s_sb[:, 0:H], ps_m[:, 0:H],
                                            1e-30)
                r_sb = spool.tile([P, 4], f32, name="rsb", tag="rsb")
                nc.vector.reciprocal(r_sb[:, 0:H], s_sb[:, 0:H])
                mn = mpool.tile([P, 256], f32, name="mn", tag="mn")
                rb = r_sb[:]
                pm = ps_m[:]
                nc.vector.tensor_tensor(
                    out=mn[:].rearrange("p (d h) -> p d h", h=H),
                    in0=AP(pm.tensor, pm.offset + H,
                           [pm.ap[0], [H, DH], [1, H]]),
                    in1=AP(rb.tensor, rb.offset, [rb.ap[0], [0, DH], [1, H]]),
                    op=mybir.AluOpType.mult)
                mb = mpool.tile([P, 256], f32, name="mb", tag="mb")
                nc.vector.tensor_tensor(out=mb[:], in0=mn[:], in1=b_tile[:],
                                        op=mybir.AluOpType.add)
                if l < 2:
                    hb = mpool.tile([P, 256], bf16, name="hb", tag="hb")
                    nc.scalar.activation(hb[:], mb[:], AF.Relu)
                    hTt = tpool.tile([P, 2 * P], bf16, name="hTt", tag="hTt")
                    for k in range(2):
                        pt = psum_tr.tile([P, P], bf16, name="pstr",
                                          tag="pstr")
                        nc.tensor.transpose(pt[:], hb[:, k * P:(k + 1) * P],
                                            ident[:])
                        nc.vector.tensor_copy(hTt[:, k * P:(k + 1) * P],
                                              pt[:])
                    nxt = hT_dram[l + 1]
                    nap = nxt[:, t * P:(t + 1) * P]
                    nc.sync.dma_start(
                        AP(nap.tensor, nap.offset,
                           [[nsp, P], [P * nsp, 2], [1, P]]),
                        hTt[:].rearrange("p (k q) -> p k q", k=2))
                else:
                    nc.sync.dma_start(out_d[t * P:(t + 1) * P, :], mb[:])

    nc.compile()
    return nc


def _make_in_maps(feats, wpacks, biases, pos, idx16, ns, nsp, in_dim):
    bf = ml_dtypes.bfloat16
    in_maps = []
    for c in range(NSH):
        sl = np.zeros((nsp, in_dim), np.float32)
        sl[pos[c, :ns]] = feats[c * ns:(c + 1) * ns]
        in_maps.append({
            "h0T": np.ascontiguousarray(sl.T).astype(bf),
            "wpack0": wpacks[0].astype(bf),
            "wpack1": wpacks[1].astype(bf),
            "wpack2": wpacks[2].astype(bf),
            "biases": biases.astype(np.float32),
            "sentinel": np.full(4, -1e30, np.float32).view(
                ml_dtypes.bfloat16).reshape(1, 8),
            "src_idx": np.ascontiguousarray(idx16[c]),
        })
    return in_maps


def gat_host(feats, src, dst, W0, al0, ar0, b0, W1, al1, ar1, b1,
             W2, al2, ar2, b2, ns=NS, nsp=NSP, in_dim=IN_DIM, run=None):
    """Full host flow: preprocess, build, run (via `run` callback), unshard."""
    feats = np.asarray(feats, np.float32)
    heads = [al0.shape[0], al1.shape[0], al2.shape[0]]
    perm01 = _dmaj(heads[0])
    perm12 = _dmaj(heads[1])
    wpacks = [pack_weights(W0, al0, ar0),
              pack_weights(W1, al1, ar1, in_perm=perm01),
              pack_weights(W2, al2, ar2, in_perm=perm12)]
    biases = np.stack([np.asarray(b0, np.float32)[_dmaj(heads[0])],
                       np.asarray(b1, np.float32)[_dmaj(heads[1])],
                       np.asarray(b2, np.float32)[_dmaj(heads[2])]])
    nb, base, B, pos, idx16 = preprocess_edges(src, dst, ns, nsp, NSH)
    nc = build_bass(nsp, in_dim, nb, base, B, heads)
    in_maps = _make_in_maps(feats, wpacks, biases, pos, idx16, ns, nsp,
                            in_dim)
    results = run(nc, in_maps)
    out = np.concatenate(
        [results[c]["out"][pos[c, :ns]] for c in range(NSH)], axis=0)
    return np.ascontiguousarray(out.astype(np.float32))


def kernel(**inputs):
    from concourse.bass_utils import run_bass_kernel_spmd

    trace = os.environ.get("GAT_TRACE", "0") == "1"
    tmpdir = os.environ.get("GAT_TRACE_DIR") or None

    def run(nc, in_maps):
        res = run_bass_kernel_spmd(nc, in_maps, core_ids=list(range(NSH)),
                                   trace=trace, tmpdir=tmpdir)
        if trace:
            print(f"HW exec time: {res.exec_time_ns} ns")
        return res.results

    return gat_host(
        inputs["feats"], inputs["src"], inputs["dst"],
        inputs["W0"], inputs["al0"], inputs["ar0"], inputs["b0"],
        inputs["W1"], inputs["al1"], inputs["ar1"], inputs["b1"],
        inputs["W2"], inputs["al2"], inputs["ar2"], inputs["b2"],
        run=run)


# revision 27
# speedup vs baseline: 1.8196x; 1.1141x over previous
"""3-layer GAT (DGL-style GATConv) on one TRN2 chip (8 NeuronCores).

Sharding: nodes are range-partitioned across the 8 cores (graph parallel).
Within a core, nodes are permuted by descending in-degree and assigned to
(tile, partition) slots so that every destination node owns one partition
row: edge slot (p, b) of tile t holds the b-th incoming edge of the node at
rank t*128+p.  This makes the edge softmax + aggregation selector-free:

  - er broadcast is a stride-0 AP read of the tile's own er column,
  - the weighted-message reduction over b is a chain of identity-stationary
    matmuls accumulating in PSUM (the exp(t) values ride along as H extra
    columns and produce the softmax denominators in the same pass),
  - padded slots gather a sentinel record whose el is -1e30 so exp()==0.

Per layer, each core packs [feat | el | er] = h @ wpack for its own nodes
(attention dots folded into the weight matrix host-side, feat columns stored
d-major so the t*feat multiply runs in the DVE 2x perf mode), the slices are
AllGather'ed, and per-edge source records are fetched from the gathered
table with one batched indirect DMA per destination tile.
"""

import os
import sys

import numpy as np

if "/opt/trn_rl_repo" not in sys.path:
    sys.path.insert(0, "/opt/trn_rl_repo")

import ml_dtypes

P = 128            # partitions / block size
NSH = 8            # shards (NeuronCores)
REC = 384          # record stride in bf16 units (768B, dma_gather needs
                   # a 256B multiple): 256 feat bf16 + 4 el f32 + pad
RECF = REC // 2    # record width in f32 units
ELF = 128          # f32-unit offset of el inside a record
PKW = 264          # packed-projection width: [feat 256 | el H | er H]

# problem constants
N, E = 50000, 800000
IN_DIM, HID, HEADS, OUT_DIM = 512, 256, 4, 256
NEG_SLOPE = 0.2
NS = N // NSH                      # 6250 real nodes per shard
TILES = (NS + P - 1) // P          # 49
NSP = TILES * P                    # 6272 padded nodes per shard
CHA = 3072                         # AllGather chunk A: ranks [0, 3072)
CHB = NSP - CHA                    # chunk B: ranks [3072, 6272)
MID = NSH * CHA                    # gather table base row (= chunk B start):
                                   # int16 idx covers the whole table
SENT_IDX = 4 * CHB + (NSP - 1 - CHA)  # core 4's lowest-degree rank, a
                                   # padding node whose el is forced to
                                   # -1e30 (positive idx, so sentinel-padded
                                   # chunk tails never trip the ucode's
                                   # trailing-negative truncation)
CHUNK = 8                          # blocks per dma_gather: 1024 descriptors
                                   # fills the SWDGE ring exactly


def _chunks(nblk):
    return [(jb, min(jb + CHUNK, nblk)) for jb in range(0, nblk, CHUNK)]


def _reserved(nblk):
    """Slot j values on partition 127 kept as sentinel (each chunk's final
    stream slot must hold a non-negative index)."""
    return {je - 1 for _, je in _chunks(nblk)}


def preprocess_edges(src, dst, ns=NS, nsp=NSP, nsh=NSH):
    """Bucket edges by dst shard; per shard, rank nodes by descending
    in-degree (rank = tile*128 + partition).  Edge slot (p, base[t]+b) holds
    the b-th incoming edge of rank t*128+p; unused slots point at SENT_IDX.

    Returns (nb[t] blocks per tile, base, B total blocks, pos[nsh, nsp]
    node->rank maps, idx16[nsh, P, B*8] per-chunk 16-partition-wrapped int16
    indices (row - MID) into the gathered table)."""
    tiles = nsp // P
    src = np.asarray(src).astype(np.int64)
    dst = np.asarray(dst).astype(np.int64)
    s_sh = src // ns
    s_loc = src - s_sh * ns
    d_sh = dst // ns
    d_loc = dst - d_sh * ns

    pos = np.zeros((nsh, nsp), np.int64)
    deg_rank = np.zeros((nsh, nsp), np.int64)
    for c in range(nsh):
        deg = np.bincount(d_loc[d_sh == c], minlength=ns)
        degf = np.concatenate([deg, np.zeros(nsp - ns, np.int64)])
        order = np.argsort(-degf, kind="stable")
        p = np.empty(nsp, np.int64)
        p[order] = np.arange(nsp)
        pos[c] = p
        deg_rank[c] = degf[order]

    maxdeg = np.maximum(1, deg_rank[:, ::P].max(axis=0))
    deg127 = deg_rank[:, P - 1::P].max(axis=0)
    nb = np.zeros(tiles, np.int64)
    for t in range(tiles):
        m = int(maxdeg[t])
        while m - len(_reserved(m)) < int(deg127[t]):
            m += 1
        nb[t] = m
    base = np.zeros(tiles, np.int64)
    base[1:] = np.cumsum(nb)[:-1]
    B = int(nb.sum())

    # table row of (core c, rank r): chunk-A rows first (c*CHA + r), then
    # chunk-B rows (MID + c*CHB + (r - CHA)); idx is relative to MID
    r_src = pos[s_sh, s_loc]
    src_row = np.where(r_src < CHA, s_sh * CHA + r_src,
                       MID + s_sh * CHB + (r_src - CHA)).astype(np.int64)
    vals = np.full((nsh, P, B), SENT_IDX, np.int16)
    allowed127 = [np.array(sorted(set(range(int(nb[t])))
                                  - _reserved(int(nb[t]))), np.int64)
                  for t in range(tiles)]
    for c in range(nsh):
        m = d_sh == c
        r = pos[c, d_loc[m]]
        sr = (src_row[m] - MID).astype(np.int16)
        o = np.argsort(r, kind="stable")
        rs, srs = r[o], sr[o]
        b = np.arange(len(rs)) - np.searchsorted(rs, rs)
        t_arr, p_arr = rs // P, rs % P
        j_arr = b.copy()
        is127 = p_arr == P - 1
        for t in range(tiles):
            sel = is127 & (t_arr == t)
            if sel.any():
                j_arr[sel] = allowed127[t][b[sel]]
        vals[c, p_arr, base[t_arr] + j_arr] = srs

    # wrap each gather chunk's slot stream (i = j*128 + p) into the 16-row
    # replicated int16 layout: value for stream slot i goes to
    # [16*g + i % 16, colbase + i // 16] for every gpsimd core g
    idx16 = np.zeros((nsh, P, B * 8), np.int16)
    for t in range(tiles):
        for jb, je in _chunks(int(nb[t])):
            blk = vals[:, :, base[t] + jb:base[t] + je]      # [nsh, P, nbk]
            stream = blk.transpose(0, 2, 1).reshape(nsh, -1)  # [nsh, i]
            w = stream.reshape(nsh, -1, 16).transpose(0, 2, 1)
            colb = int(base[t] + jb) * 8
            for g in range(8):
                idx16[:, 16 * g:16 * (g + 1), colb:colb + w.shape[2]] = w
            assert (stream[:, -1] >= 0).all(), "chunk tail must be >= 0"
    return nb, base, B, pos, idx16


def _dmaj(H, width=256):
    """Column permutation old(h-major) -> new(d-major): new j holds head
    j%H, dim j//H."""
    j = np.arange(width)
    return (j % H) * (width // H) + j // H


def pack_weights(W, al, ar, in_perm=None):
    """[W(d-major cols) | W@blockdiag(al) | W@blockdiag(ar)] -> [k, REC] f32.
    in_perm permutes W's rows to match the previous layer's d-major output."""
    W = np.asarray(W, np.float32)
    al = np.asarray(al, np.float32)
    ar = np.asarray(ar, np.float32)
    H, D = al.shape
    if in_perm is not None:
        W = W[in_perm]
    k = W.shape[0]
    W3 = W.reshape(k, H, D)
    Wel = np.einsum("khd,hd->kh", W3, al)
    Wer = np.einsum("khd,hd->kh", W3, ar)
    pad = np.zeros((k, PKW - 256 - 2 * H), np.float32)
    return np.concatenate([W[:, _dmaj(H)], Wel, Wer, pad], axis=1)


def build_bass(nsp, in_dim, nb, base, B, heads):
    """Build the 3-layer SPMD Bass graph (one graph, 8 cores)."""
    from contextlib import ExitStack

    import concourse.bacc as bacc
    import concourse.bass as bass
    import concourse.mybir as mybir
    import concourse.tile as tile
    from concourse.bass import AP, IndirectOffsetOnAxis
    from concourse.masks import make_identity

    dt = mybir.dt
    f32, bf16, i32 = dt.float32, dt.bfloat16, dt.int32
    AF = mybir.ActivationFunctionType
    tiles = nsp // P
    kdims = [in_dim, 256, 256]
    NBH = int(max(nb))

    i16 = dt.int16
    nc = bacc.Bacc("TRN2", target_bir_lowering=False, debug=False,
                   num_devices=NSH, num_swdge_queues=2)

    h0T = nc.dram_tensor("h0T", [in_dim, nsp], bf16, kind="ExternalInput")
    wps = [nc.dram_tensor(f"wpack{l}", [kdims[l], PKW], bf16,
                          kind="ExternalInput") for l in range(3)]
    bias_d = nc.dram_tensor("biases", [3, 256], f32, kind="ExternalInput")
    sent_d = nc.dram_tensor("sentinel", [1, 8], bf16, kind="ExternalInput")
    src_idx_d = nc.dram_tensor("src_idx", [P, B * 8], i16,
                               kind="ExternalInput")
    out_d = nc.dram_tensor("out", [nsp, 256], f32, kind="ExternalOutput")

    p_slice = [nc.dram_tensor(f"pslice{l}", [nsp, REC], bf16)
               for l in range(3)]
    p_full = [nc.dram_tensor(f"pfull{l}", [NSH * nsp, REC], bf16,
                             addr_space="Shared") for l in range(3)]
    skip_ag = os.environ.get("GAT_SKIP_AG", "0") == "1"

    with tile.TileContext(nc) as tc, ExitStack() as ctx:
        const = ctx.enter_context(tc.tile_pool(name="const", bufs=1))
        psum_pk = ctx.enter_context(
            tc.tile_pool(name="psum_pk", bufs=2, space="PSUM"))
        psum_ms = ctx.enter_context(
            tc.tile_pool(name="psum_ms", bufs=2, space="PSUM"))
        psum_tr = ctx.enter_context(
            tc.tile_pool(name="psum_tr", bufs=2, space="PSUM"))
        gpool = ctx.enter_context(tc.tile_pool(name="gpool", bufs=2))
        rpool = ctx.enter_context(tc.tile_pool(name="rpool", bufs=2))
        hpool = ctx.enter_context(tc.tile_pool(name="hpool", bufs=2))
        ppool = ctx.enter_context(tc.tile_pool(name="ppool", bufs=2))
        spool = ctx.enter_context(tc.tile_pool(name="spool", bufs=4))
        mpool = ctx.enter_context(tc.tile_pool(name="mpool", bufs=2))
        tpool = ctx.enter_context(tc.tile_pool(name="tpool", bufs=2))

        ident = const.tile([P, P], bf16, name="ident", tag="ident")
        make_identity(nc, ident[:])
        src_idx_sb = const.tile([P, B * 8], i16, name="srcidx", tag="srcidx")
        nc.sync.dma_start(src_idx_sb[:], src_idx_d[:, :])
        qn = [0]
        ag_sem = [nc.alloc_semaphore(f"agsem{l}") for l in range(3)]

        w_sb, b_tile, er_all = [], [], []
        for l in range(3):
            kch = kdims[l] // P
            ws = [const.tile([P, PKW], bf16, name=f"w{l}_{k}",
                             tag=f"w{l}_{k}") for k in range(kch)]
            for k in range(kch):
                nc.sync.dma_start(ws[k][:], wps[l][k * P:(k + 1) * P, :])
            w_sb.append(ws)
            bt = const.tile([P, 256], f32, name=f"btile{l}", tag=f"btile{l}")
            nc.sync.dma_start(bt[:], bias_d[l:l + 1, :].to_broadcast(
                (P, 256)))
            b_tile.append(bt)
            er_all.append(const.tile([P, tiles * heads[l]], f32,
                                     name=f"erall{l}", tag=f"erall{l}"))

        def emit_ag(l, chunk):
            """AllGather one rank-range chunk of the packed table."""
            if skip_ag:
                return
            r0, r1, o0 = (0, CHA, 0) if chunk == 0 else (CHA, nsp, NSH * CHA)
            nc.gpsimd.collective_compute(
                "AllGather", mybir.AluOpType.bypass,
                replica_groups=[list(range(NSH))],
                ins=[p_slice[l][r0:r1, :].opt()],
                outs=[p_full[l][o0:o0 + NSH * (r1 - r0), :].opt()])

        def emit_pack(l, t, hin):
            """[feat | el | er] = h @ wpack for tile t of layer l."""
            H = heads[l]
            kch = kdims[l] // P
            ps = psum_pk.tile([P, PKW], f32, name="pspk", tag="pspk")
            for k in range(kch):
                nc.tensor.matmul(
                    ps[:], lhsT=hin[:, k * P:(k + 1) * P],
                    rhs=w_sb[l][k][:], start=(k == 0), stop=(k == kch - 1))
            pack = ppool.tile([P, REC], bf16, name="pack", tag="pack")
            nc.scalar.activation(pack[:, 0:256], ps[:, 0:256], AF.Copy)
            pf = pack[:].bitcast(f32)
            el_dst = AP(pf.tensor, pf.offset + ELF, [pf.ap[0], [1, H]])
            nc.vector.tensor_copy(el_dst, ps[:, 256:256 + H])
            nc.vector.tensor_copy(er_all[l][:, t * H:(t + 1) * H],
                                  ps[:, 256 + H:256 + 2 * H])
            nc.sync.dma_start(p_slice[l][t * P:(t + 1) * P, :], pack[:])
            if t == CHA // P - 1:
                emit_ag(l, 0)

        def finish_pack(l):
            # sentinel record: el = -1e30 so exp(lrelu(el+er)) == 0
            H = heads[l]
            nc.sync.dma_start(p_slice[l][nsp - 1:nsp, 256:256 + 2 * H],
                              sent_d[0:1, 0:2 * H])
            emit_ag(l, 1)

        # ---- layer-0 pack (from the DRAM input) ----
        for t in range(tiles):
            hch = hpool.tile([P, (in_dim // P) * P], bf16, name="hch",
                             tag="hch")
            hap = h0T[:, t * P:(t + 1) * P]
            nc.sync.dma_start(
                hch[:].rearrange("p (k q) -> p k q", k=in_dim // P),
                AP(hap.tensor, hap.offset,
                   [[nsp, P], [P * nsp, in_dim // P], [1, P]]))
            emit_pack(0, t, hch[:])
        finish_pack(0)

        for l in range(3):
            H = heads[l]
            DH = 256 // H
            CH = 256 + H           # fold columns: [sum | msg]

            # ---- edge phase (fused with next layer's pack) ----
            # The gathers' declared in_ap only covers chunk B (reads of
            # chunk A go through negative offsets, invisible to the dep
            # tracker), so chain chunk A's AllGather to the gathers with a
            # touch DMA + semaphore.
            if not skip_ag and os.environ.get("GAT_NO_TOUCH", "0") != "1":
                touch = spool.tile([1, 8], bf16, name="touch", tag="touch")
                nc.sync.dma_start(touch[:], p_full[l][0:1, 0:8]).then_inc(
                    ag_sem[l], 16)
                nc.gpsimd.wait_ge(ag_sem[l], 16)
            for t in range(tiles):
                nblk = int(nb[t])
                g0 = int(base[t])

                G = gpool.tile([P, NBH * REC], bf16, name="G", tag="G")
                for jb, je in _chunks(nblk):
                    nidx = (je - jb) * P
                    colb = (g0 + jb) * 8
                    nc.gpsimd.dma_gather(
                        out_ap=G[:, jb * REC:je * REC].rearrange(
                            "p (b e) -> p b e", e=REC),
                        in_ap=p_full[l][MID:NSH * nsp, :],
                        idxs_ap=src_idx_sb[:, colb:colb + nidx // 16],
                        num_idxs=nidx,
                        num_idxs_reg=nidx,
                        elem_size=REC,
                        queue_num=qn[0])
                    qn[0] ^= 1

                # e = lrelu(el + er); t = exp(e)
                gf = G[:].bitcast(f32)
                el_ap = AP(gf.tensor, gf.offset + ELF,
                           [gf.ap[0], [RECF, nblk], [1, H]])
                ea = er_all[l][:]
                er_ap = AP(ea.tensor, ea.offset + t * H,
                           [ea.ap[0], [0, nblk], [1, H]])
                e1 = spool.tile([P, NBH * H], f32, name="e1", tag="e1")
                e13 = e1[:, 0:nblk * H].rearrange("p (b h) -> p b h", h=H)
                nc.vector.tensor_tensor(out=e13, in0=el_ap, in1=er_ap,
                                        op=mybir.AluOpType.add)
                e2 = spool.tile([P, NBH * H], f32, name="e2", tag="e2")
                nc.vector.tensor_scalar_mul(
                    e2[:, 0:nblk * H], e1[:, 0:nblk * H], NEG_SLOPE)
                e3 = spool.tile([P, NBH * H], f32, name="e3", tag="e3")
                nc.vector.tensor_tensor(out=e3[:, 0:nblk * H],
                                        in0=e1[:, 0:nblk * H],
                                        in1=e2[:, 0:nblk * H],
                                        op=mybir.AluOpType.max)
                t_bf = spool.tile([P, NBH * 4], bf16, name="tbf", tag="tbf")
                nc.scalar.activation(t_bf[:, 0:nblk * H], e3[:, 0:nblk * H],
                                     AF.Exp)
                tb = t_bf[:]
                if H == 1:
                    # replicate t x4 so the big multiply gets an innermost
                    # unit-stride run (DVE 2x perf mode)
                    t4 = spool.tile([P, NBH * 4], bf16, name="t4", tag="t4")
                    nc.vector.tensor_copy(
                        t4[:, 0:nblk * 4].rearrange("p (b r) -> p b r", r=4),
                        AP(tb.tensor, tb.offset, [tb.ap[0], [1, nblk],
                                                  [0, 4]]))
                    tin, tw = t4[:], 4
                else:
                    tin, tw = tb, H

                # rhs = [t | t * feat] per block (feat d-major)
                rhs = rpool.tile([P, NBH * CH], bf16, name="rhs", tag="rhs")
                rap = rhs[:]
                nc.vector.tensor_copy(
                    AP(rap.tensor, rap.offset, [rap.ap[0], [CH, nblk],
                                                [1, H]]),
                    AP(tb.tensor, tb.offset, [tb.ap[0], [H, nblk], [1, H]]))
                gap = G[:]
                nd = 256 // tw
                nc.vector.tensor_tensor(
                    out=AP(rap.tensor, rap.offset + H,
                           [rap.ap[0], [CH, nblk], [tw, nd], [1, tw]]),
                    in0=AP(gap.tensor, gap.offset,
                           [gap.ap[0], [REC, nblk], [tw, nd], [1, tw]]),
                    in1=AP(tin.tensor, tin.offset,
                           [tin.ap[0], [tw, nblk], [0, nd], [1, tw]]),
                    op=mybir.AluOpType.mult)

                # fold blocks: PSUM-accumulating identity matmuls
                ps_m = psum_ms.tile([P, CH], f32, name="psms", tag="psms")
                for j in range(nblk):
                    nc.tensor.matmul(
                        ps_m[:], lhsT=ident[:],
                        rhs=rhs[:, j * CH:(j + 1) * CH],
                        start=(j == 0), stop=(j == nblk - 1))

                # ---- tile epilogue: msg / sum + bias (+relu, transpose) ----
                s_sb = spool.tile([P, 4], f32, name="ssb", tag="ssb")
                nc.vector.tensor_scalar_max(s_sb[:, 0:H], ps_m[:, 0:H],
                                            1e-30)
                r_sb = spool.tile([P, 4], f32, name="rsb", tag="rsb")
                nc.vector.reciprocal(r_sb[:, 0:H], s_sb[:, 0:H])
                mn = mpool.tile([P, 256], f32, name="mn", tag="mn")
                rb = r_sb[:]
                pm = ps_m[:]
                nc.vector.tensor_tensor(
                    out=mn[:].rearrange("p (d h) -> p d h", h=H),
                    in0=AP(pm.tensor, pm.offset + H,
                           [pm.ap[0], [H, DH], [1, H]]),
                    in1=AP(rb.tensor, rb.offset, [rb.ap[0], [0, DH], [1, H]]),
                    op=mybir.AluOpType.mult)
                mb = mpool.tile([P, 256], f32, name="mb", tag="mb")
                nc.vector.tensor_tensor(out=mb[:], in0=mn[:],
                                        in1=b_tile[l][:],
                                        op=mybir.AluOpType.add)
                if l < 2:
                    hb = mpool.tile([P, 256], bf16, name="hb", tag="hb")
                    nc.scalar.activation(hb[:], mb[:], AF.Relu)
                    hTt = tpool.tile([P, 2 * P], bf16, name="hTt", tag="hTt")
                    for k in range(2):
                        pt = psum_tr.tile([P, P], bf16, name="pstr",
                                          tag="pstr")
                        nc.tensor.transpose(pt[:], hb[:, k * P:(k + 1) * P],
                                            ident[:])
                        nc.vector.tensor_copy(hTt[:, k * P:(k + 1) * P],
                                              pt[:])
                    emit_pack(l + 1, t, hTt[:])
                else:
                    nc.sync.dma_start(out_d[t * P:(t + 1) * P, :], mb[:])
            if l < 2:
                finish_pack(l + 1)

    nc.compile()
    return nc


def _make_in_maps(feats, wpacks, biases, pos, idx16, ns, nsp, in_dim):
    bf = ml_dtypes.bfloat16
    in_maps = []
    for c in range(NSH):
        sl = np.zeros((nsp, in_dim), np.float32)
        sl[pos[c, :ns]] = feats[c * ns:(c + 1) * ns]
        in_maps.append({
            "h0T": np.ascontiguousarray(sl.T).astype(bf),
            "wpack0": wpacks[0].astype(bf),
            "wpack1": wpacks[1].astype(bf),
            "wpack2": wpacks[2].astype(bf),
            "biases": biases.astype(np.float32),
            "sentinel": np.full(4, -1e30, np.float32).view(
                ml_dtypes.bfloat16).reshape(1, 8),
            "src_idx": np.ascontiguousarray(idx16[c]),
        })
    return in_maps


def gat_host(feats, src, dst, W0, al0, ar0, b0, W1, al1, ar1, b1,
             W2, al2, ar2, b2, ns=NS, nsp=NSP, in_dim=IN_DIM, run=None):
    """Full host flow: preprocess, build, run (via `run` callback), unshard."""
    feats = np.asarray(feats, np.float32)
    heads = [al0.shape[0], al1.shape[0], al2.shape[0]]
    perm01 = _dmaj(heads[0])
    perm12 = _dmaj(heads[1])
    wpacks = [pack_weights(W0, al0, ar0),
              pack_weights(W1, al1, ar1, in_perm=perm01),
              pack_weights(W2, al2, ar2, in_perm=perm12)]
    biases = np.stack([np.asarray(b0, np.float32)[_dmaj(heads[0])],
                       np.asarray(b1, np.float32)[_dmaj(heads[1])],
                       np.asarray(b2, np.float32)[_dmaj(heads[2])]])
    nb, base, B, pos, idx16 = preprocess_edges(src, dst, ns, nsp, NSH)
    nc = build_bass(nsp, in_dim, nb, base, B, heads)
    in_maps = _make_in_maps(feats, wpacks, biases, pos, idx16, ns, nsp,
                            in_dim)
    results = run(nc, in_maps)
    out = np.concatenate(
        [results[c]["out"][pos[c, :ns]] for c in range(NSH)], axis=0)
    return np.ascontiguousarray(out.astype(np.float32))


def kernel(**inputs):
    from concourse.bass_utils import run_bass_kernel_spmd

    trace = os.environ.get("GAT_TRACE", "0") == "1"
    tmpdir = os.environ.get("GAT_TRACE_DIR") or None

    def run(nc, in_maps):
        res = run_bass_kernel_spmd(nc, in_maps, core_ids=list(range(NSH)),
                                   trace=trace, tmpdir=tmpdir)
        if trace:
            print(f"HW exec time: {res.exec_time_ns} ns")
        return res.results

    return gat_host(
        inputs["feats"], inputs["src"], inputs["dst"],
        inputs["W0"], inputs["al0"], inputs["ar0"], inputs["b0"],
        inputs["W1"], inputs["al1"], inputs["ar1"], inputs["b1"],
        inputs["W2"], inputs["al2"], inputs["ar2"], inputs["b2"],
        run=run)
